# revision 15
# baseline (speedup 1.0000x reference)
"""PEER (product-key expert retrieval) CMix block on 8 trn2 NeuronCores.

Strategy: data-parallel over tokens (256 tokens/core), expert tables
replicated. Everything dense on the TensorEngine:
  - xk = token-shift mix (DVE)
  - q^T = Wq^T-layout matmul; sim via per-(p,h) small matmuls
  - top-8 routing with native vector.max / max_index
  - expert scores h = xk @ w_down^T computed DENSE (4096 experts; avg
    selection reuse ~32x makes dense cheaper than gathers)
  - softmax gate scattered per-token into a dense [t, 4096] gate row
    (local_scatter, bf16 hi+lo split for f32 accuracy, duplicates merged
    on-DVE), multiplied by tanh-gelu(h), PE-transposed to [e, t] and fed
    as lhsT into the dense up-projection gate^T-matmul against w_up.
"""
import sys
if '/opt/trn_rl_repo' not in sys.path:
    sys.path.insert(0, '/opt/trn_rl_repo')

import numpy as np
from contextlib import ExitStack

import concourse.bass as bass
import concourse.bacc as bacc
import concourse.mybir as mybir
import concourse.tile as tile
from concourse.bass_utils import run_bass_kernel_spmd
from concourse.masks import make_identity

F32 = mybir.dt.float32
I32 = mybir.dt.int32
U32 = mybir.dt.uint32
I16 = mybir.dt.int16
BF16 = mybir.dt.bfloat16
Alu = mybir.AluOpType
Act = mybir.ActivationFunctionType
AX = mybir.AxisListType

B, T, D = 2, 1024, 1024
H, NK, DK, K = 8, 64, 512, 8
NE = NK * NK            # 4096
NCORES = 8
TC = (B * T) // NCORES  # 256 tokens per core
NT = TC // 128          # 2 token chunks of 128

GELU_C = float(np.sqrt(2.0 / np.pi))


def build_program():
    nc = bacc.Bacc("TRN2", target_bir_lowering=False, debug=False,
                   num_devices=NCORES)
    xs = nc.declare_dram_parameter("xs", [TC, D], F32, isOutput=False)
    xps = nc.declare_dram_parameter("xps", [TC, D], F32, isOutput=False)
    tmk = nc.declare_dram_parameter("tmk", [1, D], F32, isOutput=False)
    # routing matrix: wqk[d', (p,h,k)] = sum_d Wq[d', phd] * keys[h,k,p,d]
    wqk = nc.declare_dram_parameter("wqk", [D, 2 * H * NK], F32, isOutput=False)
    wdT = nc.declare_dram_parameter("wdT", [D, NE], F32, isOutput=False)
    wup = nc.declare_dram_parameter("wup", [NE, D], F32, isOutput=False)
    y = nc.declare_dram_parameter("y", [TC, D], F32, isOutput=True)

    with tile.TileContext(nc) as tc_, ExitStack() as ctx:
        emit(nc, tc_, ctx, xs, xps, tmk, wqk, wdT, wup, y)
    nc.compile()
    return nc


def emit(nc, tc_, ctx, xs, xps, tmk, wqk, wdT, wup, y):
    # ---- long-lived sbuf pools ----
    consts = ctx.enter_context(tc_.tile_pool(name="consts", bufs=1))
    acts = ctx.enter_context(tc_.tile_pool(name="acts", bufs=1))
    routb = ctx.enter_context(tc_.tile_pool(name="routb", bufs=1))
    gate = ctx.enter_context(tc_.tile_pool(name="gate", bufs=1))

    ident = consts.tile([128, 128], F32)
    make_identity(nc, ident[:])
    tmk_sb = consts.tile([128, D], F32)
    nc.sync.dma_start(tmk_sb[:], tmk[:].to_broadcast([128, D]))
    iota8 = consts.tile([128, 8], I32)
    nc.gpsimd.iota(iota8[:], pattern=[[1, 8]], base=0, channel_multiplier=0)
    iota8f = consts.tile([128, 8], F32)
    nc.vector.tensor_copy(iota8f[:], iota8[:])
    iota64 = consts.tile([128, 64], I32)
    nc.gpsimd.iota(iota64[:], pattern=[[1, 64]], base=0, channel_multiplier=0)
    iota64f = consts.tile([128, 64], F32)
    nc.vector.tensor_copy(iota64f[:], iota64[:])
    iotaJ = consts.tile([128, 64], I32)   # j + 1000 for min-position dedup
    nc.gpsimd.iota(iotaJ[:], pattern=[[1, 64]], base=1000, channel_multiplier=0)
    iotaJf = consts.tile([128, 64], F32)
    nc.vector.tensor_copy(iotaJf[:], iotaJ[:])

    xk_sb = acts.tile([128, NT, D], F32)       # xk, token-major
    xkT_sb = acts.tile([128, 8, TC], F32)      # xk^T: [d%128, d//128, t]
    sim_sb = acts.tile([128, NT, 2, H, NK], F32)
    gelu_sb = acts.tile([128, NT, NE], F32)
    ghi_sb = gate.tile([128, NT, NE], BF16)
    glo_sb = gate.tile([128, NT, NE], BF16)

    # ================= phase 1: xk, xk^T, sim = xk @ wqk =================
    with tc_.tile_pool(name="p1", bufs=2) as p1, \
         tc_.tile_pool(name="psA", bufs=2, space="PSUM") as psA, \
         tc_.tile_pool(name="psS", bufs=2, space="PSUM") as psS:
        for t in range(NT):
            x_t = p1.tile([128, D], F32, tag="x")
            xp_t = p1.tile([128, D], F32, tag="xp")
            nc.sync.dma_start(x_t[:], xs[t * 128:(t + 1) * 128, :])
            nc.sync.dma_start(xp_t[:], xps[t * 128:(t + 1) * 128, :])
            # xk = x + (xprev - x) * tmk
            nc.vector.tensor_sub(xp_t[:], xp_t[:], x_t[:])
            nc.vector.tensor_mul(xp_t[:], xp_t[:], tmk_sb[:])
            nc.vector.tensor_add(xk_sb[:, t, :], x_t[:], xp_t[:])
            for kc in range(8):
                pt = psA.tile([128, 128], F32, tag="pA", space="PSUM")
                nc.tensor.matmul(pt[:], lhsT=xk_sb[:, t, bass.ts(kc, 128)],
                                 rhs=ident[:], is_transpose=True,
                                 start=True, stop=True)
                nc.scalar.copy(xkT_sb[:, kc, t * 128:(t + 1) * 128], pt[:])
        # sim[t, (p,h,k)] = xk @ wqk
        pss = {t: psS.tile([128, 2 * H * NK], F32, tag="pS",
                           name=f"pss_{t}", space="PSUM") for t in range(NT)}
        for kc in range(8):
            wq_t = p1.tile([128, 2 * H * NK], F32, tag="wq")
            nc.sync.dma_start(wq_t[:], wqk[bass.ts(kc, 128), :])
            for t in range(NT):
                for j in range(2):
                    nc.tensor.matmul(
                        pss[t][:, bass.ts(j, 512)],
                        lhsT=xkT_sb[:, kc, t * 128:(t + 1) * 128],
                        rhs=wq_t[:, bass.ts(j, 512)],
                        start=(kc == 0), stop=(kc == 7))
        for t in range(NT):
            nc.scalar.copy(sim_sb[:, t, :, :, :], pss[t][:])

    # ================= phase 2: routing =================
    with tc_.tile_pool(name="p2", bufs=2) as p2, \
         tc_.tile_pool(name="p2big", bufs=1) as p2big:
        for t in range(NT):
            sx8 = p2.tile([128, 2, H, 8], F32, tag="sx8")
            ix8u = p2.tile([128, 2, H, 8], U32, tag="ix8u")
            for p in range(2):
                for h in range(H):
                    nc.vector.max(sx8[:, p, h, :], sim_sb[:, t, p, h, :])
                    nc.vector.max_index(ix8u[:, p, h, :], sx8[:, p, h, :],
                                        sim_sb[:, t, p, h, :])
            ix8f = p2.tile([128, 2, H, 8], F32, tag="ix8f")
            nc.vector.tensor_copy(ix8f[:], ix8u[:])
            # pre-scale first-stage indices by NK
            nc.vector.tensor_scalar_mul(ix8f[:, 0, :, :], ix8f[:, 0, :, :],
                                        float(NK))
            # cartesian candidate scores & ids: [128, h, i, j]
            cs = p2.tile([128, H, 8, 8], F32, tag="cs")
            nc.vector.tensor_tensor(
                cs[:],
                sx8[:, 0, :, :].unsqueeze(3).to_broadcast([128, H, 8, 8]),
                sx8[:, 1, :, :].unsqueeze(2).to_broadcast([128, H, 8, 8]),
                op=Alu.add)
            ci = p2.tile([128, H, 8, 8], F32, tag="ci")
            nc.vector.tensor_tensor(
                ci[:],
                ix8f[:, 0, :, :].unsqueeze(3).to_broadcast([128, H, 8, 8]),
                ix8f[:, 1, :, :].unsqueeze(2).to_broadcast([128, H, 8, 8]),
                op=Alu.add)
            fin8 = p2.tile([128, H, 8], F32, tag="fin8")
            pk8u = p2.tile([128, H, 8], U32, tag="pk8u")
            for h in range(H):
                csh = cs[:, h, :, :].rearrange("p a b -> p (a b)")
                nc.vector.max(fin8[:, h, :], csh)
                nc.vector.max_index(pk8u[:, h, :], fin8[:, h, :], csh)
            # decode pk -> (i, j), gather ci entries via one-hot match
            pkhi = p2.tile([128, H, 8], U32, tag="pkhi")
            pklo = p2.tile([128, H, 8], U32, tag="pklo")
            nc.vector.tensor_scalar(pkhi[:], pk8u[:], 3, None,
                                    op0=Alu.logical_shift_right)
            nc.vector.tensor_scalar(pklo[:], pk8u[:], 7, None,
                                    op0=Alu.bitwise_and)
            pkhif = p2.tile([128, H, 8], F32, tag="pkhif")
            pklof = p2.tile([128, H, 8], F32, tag="pklof")
            nc.vector.tensor_copy(pkhif[:], pkhi[:])
            nc.vector.tensor_copy(pklof[:], pklo[:])
            # eqh[p,h,k,m] = (pkhi[p,h,k] == m)
            eqh = p2.tile([128, H, 8, 8], F32, tag="eqh")
            i8b = iota8f[:].unsqueeze(1).unsqueeze(1).to_broadcast([128, H, 8, 8])
            nc.vector.tensor_tensor(
                eqh[:], pkhif[:].unsqueeze(3).to_broadcast([128, H, 8, 8]),
                i8b, op=Alu.is_equal)
            prodh = p2.tile([128, H, 8, 8], F32, tag="prodh")
            nc.vector.tensor_tensor(
                prodh[:], eqh[:],
                ix8f[:, 0, :, :].unsqueeze(2).to_broadcast([128, H, 8, 8]),
                op=Alu.mult)
            eid = p2.tile([128, H, 8], F32, tag="eid")
            nc.vector.tensor_reduce(eid[:].unsqueeze(3), prodh[:],
                                    axis=AX.X, op=Alu.add)
            nc.vector.tensor_tensor(
                eqh[:], pklof[:].unsqueeze(3).to_broadcast([128, H, 8, 8]),
                i8b, op=Alu.is_equal)
            nc.vector.tensor_tensor(
                prodh[:], eqh[:],
                ix8f[:, 1, :, :].unsqueeze(2).to_broadcast([128, H, 8, 8]),
                op=Alu.mult)
            eidy = p2.tile([128, H, 8], F32, tag="eidy")
            nc.vector.tensor_reduce(eidy[:].unsqueeze(3), prodh[:],
                                    axis=AX.X, op=Alu.add)
            nc.vector.tensor_add(eid[:], eid[:], eidy[:])
            # softmax over fin8 per head (sorted desc -> col 0 is max), x0.5
            mm = p2.tile([128, H, 8], F32, tag="mm")
            nc.vector.tensor_tensor(
                mm[:], fin8[:],
                fin8[:, :, 0:1].to_broadcast([128, H, 8]), op=Alu.subtract)
            nc.scalar.activation(mm[:], mm[:], Act.Exp)
            ssum = p2.tile([128, H], F32, tag="ssum")
            nc.vector.tensor_reduce(ssum[:].unsqueeze(2), mm[:],
                                    axis=AX.X, op=Alu.add)
            rec = p2.tile([128, H], F32, tag="rec")
            nc.vector.reciprocal(rec[:], ssum[:])
            nc.vector.tensor_scalar_mul(rec[:], rec[:], 0.5)
            soft = p2.tile([128, H, 8], F32, tag="soft")
            nc.vector.tensor_tensor(
                soft[:], mm[:], rec[:].unsqueeze(2).to_broadcast([128, H, 8]),
                op=Alu.mult)
            # ---- dedup duplicate experts across heads (merge soft weights) ----
            eidf = eid[:].rearrange("p h k -> p (h k)")
            softf = soft[:].rearrange("p h k -> p (h k)")
            eq = p2big.tile([128, 64, 64], F32, tag="eq")
            nc.vector.tensor_tensor(
                eq[:], eidf.unsqueeze(1).to_broadcast([128, 64, 64]),
                eidf.unsqueeze(2).to_broadcast([128, 64, 64]), op=Alu.is_equal)
            scr = p2big.tile([128, 64, 64], F32, tag="scr")
            nc.vector.tensor_tensor(
                scr[:], eq[:], softf.unsqueeze(1).to_broadcast([128, 64, 64]),
                op=Alu.mult)
            ssumd = p2.tile([128, 64], F32, tag="ssumd")
            nc.vector.tensor_reduce(ssumd[:].unsqueeze(2), scr[:],
                                    axis=AX.X, op=Alu.add)
            # min matching position: scr = eq*(-1000) + (j+1000); reduce min
            nc.vector.scalar_tensor_tensor(
                scr[:], eq[:], -1000.0,
                iotaJf[:].unsqueeze(1).to_broadcast([128, 64, 64]),
                op0=Alu.mult, op1=Alu.add)
            minpos = p2.tile([128, 64], F32, tag="minpos")
            nc.vector.tensor_reduce(minpos[:].unsqueeze(2), scr[:],
                                    axis=AX.X, op=Alu.min)
            first = p2.tile([128, 64], F32, tag="first")
            nc.vector.tensor_tensor(first[:], minpos[:], iota64f[:],
                                    op=Alu.is_equal)
            s2 = p2.tile([128, 64], F32, tag="s2")
            nc.vector.tensor_mul(s2[:], ssumd[:], first[:])
            # em = eid if first else very negative
            em = p2.tile([128, 64], F32, tag="em")
            nc.vector.tensor_scalar(em[:], first[:], 20000.0, -20000.0,
                                    op0=Alu.mult, op1=Alu.add)
            nc.vector.tensor_add(em[:], em[:], eidf)
            # hi/lo bf16 split of s2
            shb = p2.tile([128, 64], BF16, tag="shb")
            nc.vector.tensor_copy(shb[:], s2[:])
            shf = p2.tile([128, 64], F32, tag="shf")
            nc.vector.tensor_copy(shf[:], shb[:])
            slb = p2.tile([128, 64], BF16, tag="slb")
            nc.vector.tensor_sub(shf[:], s2[:], shf[:])
            nc.vector.tensor_copy(slb[:], shf[:])
            # scatter into dense [t, 4096] gate rows, 1024-wide chunks
            for c in range(4):
                idxf = p2.tile([128, 64], F32, tag="idxf")
                nc.vector.tensor_scalar_add(idxf[:], em[:], float(-1024 * c))
                hi = p2.tile([128, 64], F32, tag="hi")
                nc.vector.tensor_scalar(hi[:], idxf[:], 1024.0, -30000.0,
                                        op0=Alu.is_ge, op1=Alu.mult)
                nc.vector.tensor_add(idxf[:], idxf[:], hi[:])
                nc.vector.tensor_scalar_max(idxf[:], idxf[:], -1.0)
                idx16 = p2.tile([128, 64], I16, tag="idx16")
                nc.vector.tensor_copy(idx16[:], idxf[:])
                nc.gpsimd.local_scatter(
                    ghi_sb[:, t, bass.ts(c, 1024)], shb[:], idx16[:],
                    channels=128, num_elems=1024, num_idxs=64)
                nc.gpsimd.local_scatter(
                    glo_sb[:, t, bass.ts(c, 1024)], slb[:], idx16[:],
                    channels=128, num_elems=1024, num_idxs=64)

    # ================= phase 3: dense h = xk @ wdT, tanh-gelu =================
    with tc_.tile_pool(name="p3", bufs=3) as p3, \
         tc_.tile_pool(name="ps3", bufs=3, space="PSUM") as ps3:
        for q in range(4):
            phs = {}
            for kc in range(8):
                wd_t = p3.tile([128, 1024], F32, tag="wd")
                nc.sync.dma_start(wd_t[:], wdT[bass.ts(kc, 128),
                                               bass.ts(q, 1024)])
                for t in range(NT):
                    if kc == 0:
                        phs[t] = ps3.tile([128, 1024], F32, tag="ph",
                                          name=f"ph_q{q}_t{t}", space="PSUM")
                    for j in range(2):
                        nc.tensor.matmul(
                            phs[t][:, bass.ts(j, 512)],
                            lhsT=xkT_sb[:, kc, t * 128:(t + 1) * 128],
                            rhs=wd_t[:, bass.ts(j, 512)],
                            start=(kc == 0), stop=(kc == 7))
            for t in range(NT):
                ph = phs[t]
                sq = p3.tile([128, 1024], F32, tag="sq")
                nc.scalar.activation(sq[:], ph[:], Act.Square)
                nc.vector.tensor_scalar(sq[:], sq[:], 0.044715, 1.0,
                                        op0=Alu.mult, op1=Alu.add)
                nc.vector.tensor_mul(sq[:], sq[:], ph[:])
                nc.scalar.activation(sq[:], sq[:], Act.Tanh, scale=GELU_C)
                nc.vector.scalar_tensor_tensor(
                    gelu_sb[:, t, bass.ts(q, 1024)], sq[:], 1.0, ph[:],
                    op0=Alu.add, op1=Alu.mult)

    # ================= phase 4: gate transpose + up-projection =================
    with tc_.tile_pool(name="p4", bufs=3) as p4, \
         tc_.tile_pool(name="ps4", bufs=2, space="PSUM") as ps4, \
         tc_.tile_pool(name="ps4u", bufs=2, space="PSUM") as ps4u:
        pup = {t: ps4u.tile([128, D], F32, tag="pu", name=f"pup_{t}",
                           space="PSUM") for t in range(NT)}
        for q in range(4):
            gtq = {}
            for t in range(NT):
                ptq = ps4.tile([128, 8, 128], F32, tag="pT",
                               name=f"ptq_q{q}_t{t}", space="PSUM")
                # gate = (hi + lo) * gelu, f32 (hi/lo bf16 sum is exact in f32)
                gf = p4.tile([128, 1024], F32, tag="gf")
                nc.vector.tensor_tensor(
                    gf[:], ghi_sb[:, t, bass.ts(q, 1024)],
                    glo_sb[:, t, bass.ts(q, 1024)], op=Alu.add)
                nc.vector.tensor_mul(gf[:], gf[:],
                                     gelu_sb[:, t, bass.ts(q, 1024)])
                for ec in range(8):
                    nc.tensor.matmul(ptq[:, ec, :],
                                     lhsT=gf[:, bass.ts(ec, 128)],
                                     rhs=ident[:], is_transpose=True,
                                     start=True, stop=True)
                gtq[t] = p4.tile([128, 8, 128], F32, tag="gt",
                                 name=f"gtq_q{q}_t{t}")
                nc.scalar.copy(gtq[t][:], ptq[:])
            for c8 in range(8):
                c = q * 8 + c8
                wu_t = p4.tile([128, D], F32, tag="wu")
                nc.sync.dma_start(wu_t[:], wup[bass.ts(c, 128), :])
                for t in range(NT):
                    for j in range(2):
                        nc.tensor.matmul(
                            pup[t][:, bass.ts(j, 512)],
                            lhsT=gtq[t][:, c8, :],
                            rhs=wu_t[:, bass.ts(j, 512)],
                            start=(c == 0), stop=(c == 31))
        for t in range(NT):
            y_t = p4.tile([128, D], F32, tag="y")
            nc.scalar.copy(y_t[:], pup[t][:])
            nc.sync.dma_start(y[t * 128:(t + 1) * 128, :], y_t[:])


def fold_routing_weights(Wq, keys):
    """wqk[d', (p,h,k)] = sum_d Wq[d', p*H*DK + h*DK + d] * keys[h,k,p,d].

    Exact weight refactoring: sim = (xk @ Wq) contracted with keys over
    the 512-wide key dim equals xk @ wqk. Done in float64 on host so the
    fold itself adds no fp32 error beyond the reference's own matmul
    rounding (which differs anyway by summation order).
    """
    Wq4 = Wq.astype(np.float64).reshape(D, 2, H, DK)
    wqk = np.einsum('aphd,hkpd->aphk', Wq4, keys.astype(np.float64))
    return np.ascontiguousarray(wqk.reshape(D, 2 * H * NK).astype(np.float32))


_prog = None
LAST_RESULT = None


def _get_prog():
    global _prog
    if _prog is None:
        _prog = build_program()
    return _prog


def kernel(x, shift_state, time_maa_k, Wq, keys, w_down, w_up):
    x = np.asarray(x, dtype=np.float32)
    shift_state = np.asarray(shift_state, dtype=np.float32)
    time_maa_k = np.asarray(time_maa_k, dtype=np.float32)
    Wq = np.asarray(Wq, dtype=np.float32)
    keys = np.asarray(keys, dtype=np.float32)
    w_down = np.asarray(w_down, dtype=np.float32)
    w_up = np.asarray(w_up, dtype=np.float32)

    nc = _get_prog()
    xf = np.ascontiguousarray(x.reshape(B * T, D))
    xprev = np.concatenate([shift_state[:, None, :], x[:, :-1, :]], axis=1)
    xpf = np.ascontiguousarray(xprev.reshape(B * T, D))
    tmk = np.ascontiguousarray(time_maa_k.reshape(1, D))
    wqk = fold_routing_weights(Wq, keys)
    wdT = np.ascontiguousarray(w_down.T)

    in_maps = []
    for c in range(NCORES):
        sl = slice(c * TC, (c + 1) * TC)
        in_maps.append(dict(
            xs=np.ascontiguousarray(xf[sl]),
            xps=np.ascontiguousarray(xpf[sl]),
            tmk=tmk, wqk=wqk, wdT=wdT, wup=w_up))
    res = run_bass_kernel_spmd(nc, in_maps, list(range(NCORES)))
    global LAST_RESULT
    LAST_RESULT = res
    kv = np.concatenate([res.results[c]["y"] for c in range(NCORES)], axis=0)
    kv = kv.reshape(B, T, D)
    new_shift = np.ascontiguousarray(x[:, -1, :])
    return kv, new_shift
